# revision 3
# baseline (speedup 1.0000x reference)
"""Bidirectional LSTM encoder (nn_EncoderRNN) on 8 Trainium2 NeuronCores.

Strategy (hardcoded for VOCAB=32000, HID=512, SEQ=2048, BATCH=32, 8 cores):
  - cores 0-3: forward LSTM, batch quarters 0..3 (8 batch rows each)
  - cores 4-7: backward LSTM (sequence reversed on host), batch quarters 0..3
  - per core: embedding rows are gathered on-device (dma_gather transpose)
    into hid-major tiles, x@wx+bias precomputed as a bf16 GEMM into a DRAM
    staging buffer, then the sequential 2048-step recurrence runs with
    wh-stationary [128,128] bf16 matmuls producing transposed gates
    [gate, batch]; hidden-state history is streamed to DRAM fp32.
  - host assembles the [32, 2048*1024] fp32 output from the 8 per-core
    histories (pure reshape/transpose unsharding).
"""
import sys
import numpy as np

sys.path.insert(0, '/opt/trn_rl_repo')

import ml_dtypes  # noqa: E402

S = 2048
BATCH = 32
B = 8            # batch rows per core
HID = 512
VOCAB = 32000
NMC = 16         # gate chunks of 128 (4H = 2048)
HB = 8           # steps per For_i iteration / history block
NG = 32          # gather/GEMM groups (512 tokens each: 64 steps x 8 batch)
N_CORES = 8

_CACHE = {}
LAST_INFO = {}


def _build():
    import concourse.bass as bass  # noqa: F401
    import concourse.mybir as mybir
    import concourse.tile as tile
    from concourse import bacc
    from concourse.bass import ds, ts

    f32, bf16, i16 = mybir.dt.float32, mybir.dt.bfloat16, mybir.dt.int16
    Sig = mybir.ActivationFunctionType.Sigmoid
    Tanh = mybir.ActivationFunctionType.Tanh
    ADD, MUL = mybir.AluOpType.add, mybir.AluOpType.mult
    MC_ORDER = [0, 4, 8, 12, 1, 5, 9, 13, 2, 6, 10, 14, 3, 7, 11, 15]

    nc = bacc.Bacc("TRN2", target_bir_lowering=False, debug=False,
                   num_devices=N_CORES)
    emb_in = nc.declare_dram_parameter("embb", [VOCAB, HID], bf16, isOutput=False)
    idx_in = nc.declare_dram_parameter("idxs", [128, S * B // 16], i16, isOutput=False)
    wxs_in = nc.declare_dram_parameter("wxs", [128, 4 * 2048], bf16, isOutput=False)
    whs_in = nc.declare_dram_parameter("whs", [128, 4 * 2048], bf16, isOutput=False)
    bias_in = nc.declare_dram_parameter("bias", [128, NMC], f32, isOutput=False)
    h0_in = nc.declare_dram_parameter("h0T", [128, 4 * B], f32, isOutput=False)
    hist_out = nc.declare_dram_parameter("hist", [128, S // HB, HB, 4, B], f32,
                                         isOutput=True)

    with tile.TileContext(nc) as tc:
        with (
            tc.tile_pool(name="const", bufs=1) as constp,
            tc.tile_pool(name="state", bufs=1) as statep,
            tc.tile_pool(name="dram", bufs=1, space="DRAM") as dramp,
            tc.tile_pool(name="gat", bufs=3) as gatp,
            tc.tile_pool(name="xts", bufs=2) as xtsp,
            tc.tile_pool(name="xin", bufs=3) as xinp,
            tc.tile_pool(name="gates", bufs=3) as gatesp,
            tc.tile_pool(name="histp", bufs=2) as histp,
            tc.tile_pool(name="psum", bufs=2, space="PSUM") as psump,
        ):
            # ---- constants ----
            wxs = constp.tile([128, 4 * 2048], bf16)
            nc.sync.dma_start(out=wxs[:, :], in_=wxs_in[:, :])
            whs = constp.tile([128, 4 * 2048], bf16)
            nc.sync.dma_start(out=whs[:, :], in_=whs_in[:, :])
            bias = constp.tile([128, NMC], f32)
            nc.sync.dma_start(out=bias[:, :], in_=bias_in[:, :])
            idxt = constp.tile([128, S * B // 16], i16)
            nc.sync.dma_start(out=idxt[:, :], in_=idx_in[:, :])

            # X staging: [p, s2, sh, mc, b] bf16
            X = dramp.tile([128, S // 2, 2, NMC, B], bf16)

            # ---- prep: gather + x@wx GEMM ----
            for g in range(NG):
                embT = gatp.tile([128, 4, 512], bf16, tag="embT")
                nc.gpsimd.dma_gather(
                    out_ap=embT[:, :, :],
                    in_ap=emb_in[:, :],
                    idxs_ap=idxt[:, ts(g, 32)],
                    num_idxs=512,
                    num_idxs_reg=512,
                    elem_size=HID,
                    transpose=True,
                )
                xts = xtsp.tile([128, 32, 2, NMC, B], bf16, tag="xts")
                for mc in range(NMC):
                    pps = psump.tile([128, 32, 2, B], f32, tag="gps", name="pps")
                    for kc in range(4):
                        nc.tensor.matmul(
                            pps[:, :, :, :],
                            wxs[:, kc * 2048 + mc * 128: kc * 2048 + (mc + 1) * 128],
                            embT[:, kc, :],
                            start=(kc == 0), stop=(kc == 3),
                        )
                    nc.vector.tensor_scalar_add(xts[:, :, :, mc, :],
                                                pps[:, :, :, :],
                                                bias[:, mc:mc + 1])
                nc.sync.dma_start(out=X[:, ts(g, 32), :, :, :],
                                  in_=xts[:, :, :, :, :])

            # ---- recurrence state ----
            hT = statep.tile([128, 4 * B], f32)
            cT = statep.tile([128, 4 * B], f32)
            hbf = statep.tile([128, 4 * B], bf16)
            nc.sync.dma_start(out=hT[:, :], in_=h0_in[:, :])
            nc.sync.dma_start(out=cT[:, :], in_=h0_in[:, :])
            nc.vector.tensor_copy(hbf[:, :], hT[:, :])

            def step(xin4, step_u, histtile):
                s2r, sh = divmod(step_u, 2)
                ps0 = psump.tile([128, 8, B], f32, tag="ps0", name="ps0")
                ps1 = psump.tile([128, 8, B], f32, tag="ps1", name="ps1")
                ps = [ps0, ps1]
                gt = gatesp.tile([128, NMC * B], f32, tag="gt")
                for n, mc in enumerate(MC_ORDER):
                    bank, sl = divmod(n, 8)
                    for kc in range(4):
                        nc.tensor.matmul(
                            ps[bank][:, sl, :],
                            whs[:, kc * 2048 + mc * 128: kc * 2048 + (mc + 1) * 128],
                            hbf[:, kc * B:(kc + 1) * B],
                            start=(kc == 0), stop=(kc == 3),
                        )
                    pre = gatesp.tile([128, B], f32, tag="pre")
                    nc.vector.tensor_tensor(pre[:, :], ps[bank][:, sl, :],
                                            xin4[:, s2r, sh, mc, :], ADD)
                    nc.scalar.activation(gt[:, mc * B:(mc + 1) * B], pre[:, :],
                                         Tanh if 8 <= mc < 12 else Sig)
                for j in range(4):
                    i_s = gt[:, (0 + j) * B:(1 + j) * B]
                    f_s = gt[:, (4 + j) * B:(5 + j) * B]
                    g_s = gt[:, (8 + j) * B:(9 + j) * B]
                    o_s = gt[:, (12 + j) * B:(13 + j) * B]
                    c_s = cT[:, j * B:(j + 1) * B]
                    h_s = hT[:, j * B:(j + 1) * B]
                    ig = gatesp.tile([128, B], f32, tag="ig")
                    nc.vector.tensor_tensor(ig[:, :], i_s, g_s, MUL)
                    nc.vector.tensor_tensor(c_s, f_s, c_s, MUL)
                    nc.vector.tensor_tensor(c_s, c_s, ig[:, :], ADD)
                    tc_s = gatesp.tile([128, B], f32, tag="tc")
                    nc.scalar.activation(tc_s[:, :], c_s, Tanh)
                    nc.vector.tensor_tensor(h_s, o_s, tc_s[:, :], MUL)
                    nc.vector.tensor_copy(hbf[:, j * B:(j + 1) * B], h_s)
                nc.vector.tensor_copy(histtile[:, step_u, :, :], hT[:, :])

            with tc.For_i(0, S // HB, 1) as hb_iv:
                xin4 = xinp.tile([128, HB // 2, 2, NMC, B], bf16, tag="xin")
                nc.sync.dma_start(out=xin4[:, :, :, :, :],
                                  in_=X[:, ds(hb_iv * (HB // 2), HB // 2), :, :, :])
                histtile = histp.tile([128, HB, 4, B], f32, tag="hist")
                for step_u in range(HB):
                    step(xin4, step_u, histtile)
                nc.sync.dma_start(out=hist_out[:, ds(hb_iv, 1), :, :, :],
                                  in_=histtile[:, :, :, :])

    nc.compile()
    return nc


def _get_nc():
    if "nc" not in _CACHE:
        _CACHE["nc"] = _build()
    return _CACHE["nc"]


def _wrap_idxs(tok_flat):
    # tok_flat: [S*B] int; value j goes to [p%16, j//16] replicated over p//16
    a = tok_flat.astype(np.int16).reshape(NG, 32, 16)      # [g, c, p16]
    a = a.transpose(2, 0, 1)                               # [p16, g, c]
    a = np.tile(a, (8, 1, 1))                              # [128, g, c]
    return np.ascontiguousarray(a.reshape(128, NG * 32))


def kernel(**inputs):
    import time
    from concourse.bass_utils import run_bass_kernel_spmd

    tokens = np.asarray(inputs["tokens"])
    h0 = np.asarray(inputs["h0"], dtype=np.float32)
    embedding = np.asarray(inputs["embedding"], dtype=np.float32)

    embb = embedding.astype(ml_dtypes.bfloat16)

    def wlay(w):
        wb = np.asarray(w, dtype=np.float32).astype(ml_dtypes.bfloat16)
        return np.ascontiguousarray(
            wb.reshape(4, 128, 2048).transpose(1, 0, 2).reshape(128, 4 * 2048))

    wxs = {0: wlay(inputs["wx_f"]), 1: wlay(inputs["wx_b"])}
    whs = {0: wlay(inputs["wh_f"]), 1: wlay(inputs["wh_b"])}
    bias = {
        0: np.ascontiguousarray(
            (np.asarray(inputs["bx_f"], np.float32)
             + np.asarray(inputs["bh_f"], np.float32)).reshape(NMC, 128).T),
        1: np.ascontiguousarray(
            (np.asarray(inputs["bx_b"], np.float32)
             + np.asarray(inputs["bh_b"], np.float32)).reshape(NMC, 128).T),
    }

    in_maps = []
    for core in range(N_CORES):
        d = core // 4          # 0 = fwd, 1 = bwd
        q = core % 4           # batch quarter
        tok = tokens[:, q * B:(q + 1) * B]
        if d == 1:
            tok = tok[::-1]
        h0q = h0[q * B:(q + 1) * B]                         # [B, 512]
        h0T = np.ascontiguousarray(
            h0q.reshape(B, 4, 128).transpose(2, 1, 0).reshape(128, 4 * B))
        in_maps.append({
            "embb": embb,
            "idxs": _wrap_idxs(np.ascontiguousarray(tok).reshape(-1)),
            "wxs": wxs[d],
            "whs": whs[d],
            "bias": bias[d],
            "h0T": h0T,
        })

    nc = _get_nc()
    t0 = time.perf_counter()
    res = run_bass_kernel_spmd(nc, in_maps, list(range(N_CORES)))
    LAST_INFO["run_wall_s"] = time.perf_counter() - t0

    # ---- unshard: hist [128, 256, 8, 4, B] -> h_loc [S, B, 512] ----
    out = np.empty((BATCH, S, 2, HID), np.float32)
    for core in range(N_CORES):
        d, q = core // 4, core % 4
        h = res.results[core]["hist"]                       # [128, 256, 8, 4, B]
        h = h.transpose(1, 2, 4, 3, 0).reshape(S, B, HID)   # [s_loc, b, hid]
        if d == 1:
            h = h[::-1]
        out[q * B:(q + 1) * B, :, d, :] = h.transpose(1, 0, 2)
    return np.ascontiguousarray(out.reshape(BATCH, S * 2 * HID))


# revision 6
# speedup vs baseline: 1.7117x; 1.7117x over previous
"""Bidirectional LSTM encoder (nn_EncoderRNN) on 8 Trainium2 NeuronCores.

Strategy (hardcoded for VOCAB=32000, HID=512, SEQ=2048, BATCH=32, 8 cores):
  - cores 0-3: forward LSTM, batch quarters 0..3 (8 batch rows each)
  - cores 4-7: backward LSTM (sequence reversed on host), batch quarters 0..3
  - per core: embedding rows gathered on-device (dma_gather transpose) into
    hid-major tiles; x@wx + bias precomputed as a bf16 GEMM into DRAM staging
    X2 [S*B, 2048] (batch-major rows, gate columns permuted to [i f o g]);
    the 2048-step recurrence keeps h^T stationary on the PE (4 LDW of
    [128,8]) and streams wh as the moving operand (16 matmuls of N=512 per
    step), injects x@wx and h-transposes via tiny identity matmuls, and runs
    batched activations (one sigmoid over [8,1536], one tanh over [8,512])
    plus 5 DVE cell ops per step. History is written batch-major fp32 so the
    host unshard is a plain slice assignment.
"""
import sys
import numpy as np

sys.path.insert(0, '/opt/trn_rl_repo')

import ml_dtypes  # noqa: E402

S = 2048
BATCH = 32
B = 8            # batch rows per core
HID = 512
VOCAB = 32000
HB = 16          # steps per For_i iteration / history block
NG = S * B // 512
N_CORES = 8

_CACHE = {}
LAST_INFO = {}

# gate-column permutation: reference order [i f g o] -> stored [i f o g]
_PERM = np.concatenate([np.arange(0, 1024), np.arange(1536, 2048),
                        np.arange(1024, 1536)])


def _build():
    import concourse.mybir as mybir
    import concourse.tile as tile
    from concourse import bacc
    from concourse.bass import ds, ts

    f32, bf16, i16 = mybir.dt.float32, mybir.dt.bfloat16, mybir.dt.int16
    Sig = mybir.ActivationFunctionType.Sigmoid
    Tanh = mybir.ActivationFunctionType.Tanh
    ADD, MUL = mybir.AluOpType.add, mybir.AluOpType.mult

    nc = bacc.Bacc("TRN2", target_bir_lowering=False, debug=False,
                   num_devices=N_CORES)
    emb_in = nc.declare_dram_parameter("embb", [VOCAB, 512], bf16, isOutput=False)
    idx_in = nc.declare_dram_parameter("idxs", [128, S * B // 16], i16, isOutput=False)
    wxs_in = nc.declare_dram_parameter("wxs", [128, 8192], bf16, isOutput=False)
    whs_in = nc.declare_dram_parameter("whs", [128, 8192], bf16, isOutput=False)
    bias_in = nc.declare_dram_parameter("biasb", [1, 2048], bf16, isOutput=False)
    h0T_in = nc.declare_dram_parameter("h0T", [128, 4 * B], f32, isOutput=False)
    h0r_in = nc.declare_dram_parameter("h0r", [B, 512], f32, isOutput=False)
    eye_in = nc.declare_dram_parameter("eye8", [B, B], bf16, isOutput=False)
    hist_out = nc.declare_dram_parameter("hist", [B, S, 512], f32, isOutput=True)

    with tile.TileContext(nc) as tc:
        with (
            tc.tile_pool(name="const", bufs=1) as constp,
            tc.tile_pool(name="state", bufs=1) as statep,
            tc.tile_pool(name="dram", bufs=1, space="DRAM") as dramp,
            tc.tile_pool(name="gat", bufs=3) as gatp,
            tc.tile_pool(name="xts", bufs=3) as xtsp,
            tc.tile_pool(name="xin", bufs=4) as xinp,
            tc.tile_pool(name="gates", bufs=3) as gatesp,
            tc.tile_pool(name="histp", bufs=2) as histp,
            tc.tile_pool(name="psA", bufs=1, space="PSUM") as psA,
            tc.tile_pool(name="psB", bufs=2, space="PSUM") as psB,
        ):
            wxs = constp.tile([128, 8192], bf16)
            nc.sync.dma_start(out=wxs[:, :], in_=wxs_in[:, :])
            whs = constp.tile([128, 8192], bf16)
            nc.sync.dma_start(out=whs[:, :], in_=whs_in[:, :])
            biasb = constp.tile([1, 2048], bf16)
            nc.sync.dma_start(out=biasb[:, :], in_=bias_in[:, :])
            idxt = constp.tile([128, S * B // 16], i16)
            nc.sync.dma_start(out=idxt[:, :], in_=idx_in[:, :])
            ones1 = constp.tile([1, 128], bf16)
            nc.vector.memset(ones1[:, :], 1.0)
            eye8 = constp.tile([B, B], bf16)
            nc.sync.dma_start(out=eye8[:, :], in_=eye_in[:, :])

            X2 = dramp.tile([S * B, 2048], bf16)

            # ---- prep: gather + x@wx GEMM (+bias) ----
            for g in range(NG):
                embT = gatp.tile([128, 4, 512], bf16, tag="embT")
                nc.gpsimd.dma_gather(
                    out_ap=embT[:, :, :],
                    in_ap=emb_in[:, :],
                    idxs_ap=idxt[:, ts(g, 32)],
                    num_idxs=512,
                    num_idxs_reg=512,
                    elem_size=512,
                    transpose=True,
                )
                for mt in range(4):
                    for nt in range(4):
                        pps = psB.tile([128, 512], f32, tag="gps", name="pps")
                        for kc in range(4):
                            nc.tensor.matmul(
                                pps[:, :],
                                embT[:, kc, ts(mt, 128)],
                                wxs[:, kc * 2048 + nt * 512: kc * 2048 + (nt + 1) * 512],
                                start=(kc == 0), stop=False,
                            )
                        nc.tensor.matmul(
                            pps[:, :], ones1[:, :], biasb[:, ts(nt, 512)],
                            start=False, stop=True,
                        )
                        xt = xtsp.tile([128, 512], bf16, tag="xt")
                        nc.vector.tensor_copy(xt[:, :], pps[:, :])
                        nc.sync.dma_start(
                            out=X2[ds(g * 512 + mt * 128, 128), ts(nt, 512)],
                            in_=xt[:, :])

            # ---- recurrence ----
            hbfT = statep.tile([128, 4 * B], bf16)   # stationary h^T (bf16)
            h0Tt = statep.tile([128, 4 * B], f32)
            nc.sync.dma_start(out=h0Tt[:, :], in_=h0T_in[:, :])
            nc.vector.tensor_copy(hbfT[:, :], h0Tt[:, :])
            cR = statep.tile([B, 512], f32)          # batch-major cell state
            nc.sync.dma_start(out=cR[:, :], in_=h0r_in[:, :])

            def step(iv, u, histtile):
                # gates psum [B, 2048] across 4 bank-tiles; cols [i f o g]
                gps = psA.tile([B, 4, 512], f32, tag="rg", name="gps")
                xin = xinp.tile([B, 2048], bf16, tag="xin")
                nc.sync.dma_start(out=xin[:, :],
                                  in_=X2[ds((iv * HB + u) * B, B), :])
                for nt in range(4):
                    for kc in range(4):
                        nc.tensor.matmul(
                            gps[:, nt, :],
                            hbfT[:, kc * B:(kc + 1) * B],
                            whs[:, kc * 2048 + nt * 512: kc * 2048 + (nt + 1) * 512],
                            start=(kc == 0), stop=False,
                        )
                    nc.tensor.matmul(
                        gps[:, nt, :], eye8[:, :],
                        xin[:, ts(nt, 512)],
                        start=False, stop=True,
                    )
                gifo = gatesp.tile([B, 1536], f32, tag="gifo")
                nc.scalar.activation(gifo[:, :], gps[:, 0:3, :], Sig)
                gg = gatesp.tile([B, 512], f32, tag="gg")
                nc.scalar.activation(gg[:, :], gps[:, 3, :], Tanh)
                # cell update (batch-major [B, 512])
                ig = gatesp.tile([B, 512], f32, tag="ig")
                nc.vector.tensor_tensor(ig[:, :], gifo[:, 0:512], gg[:, :], MUL)
                nc.vector.tensor_tensor(cR[:, :], gifo[:, 512:1024], cR[:, :], MUL)
                nc.vector.tensor_tensor(cR[:, :], cR[:, :], ig[:, :], ADD)
                tcs = gatesp.tile([B, 512], f32, tag="tcs")
                nc.scalar.activation(tcs[:, :], cR[:, :], Tanh)
                hR = histtile[:, u, :]
                nc.vector.tensor_tensor(hR, gifo[:, 1024:1536], tcs[:, :], MUL)
                hRb = gatesp.tile([B, 512], bf16, tag="hRb")
                nc.vector.tensor_tensor(hRb[:, :], gifo[:, 1024:1536], tcs[:, :], MUL)
                # transpose hRb -> hbfT via PE (4x [B,128] -> [128,B])
                tps = psB.tile([128, 4, B], f32, tag="tps", name="tps")
                for kc in range(4):
                    nc.tensor.matmul(tps[:, kc, :], hRb[:, ts(kc, 128)],
                                     eye8[:, :], start=True, stop=True)
                nc.vector.tensor_copy(hbfT[:, :], tps[:, :, :])

            with tc.For_i(0, S // HB, 1) as iv:
                histtile = histp.tile([B, HB, 512], f32, tag="hist")
                for u in range(HB):
                    step(iv, u, histtile)
                nc.sync.dma_start(out=hist_out[:, ds(iv * HB, HB), :],
                                  in_=histtile[:, :, :])

    nc.compile()
    return nc


def _get_nc():
    if "nc" not in _CACHE:
        _CACHE["nc"] = _build()
    return _CACHE["nc"]


def _wrap_idxs(tok_flat):
    # tok_flat: [S*B] int; value j goes to [p%16, j//16] replicated over p//16
    a = tok_flat.astype(np.int16).reshape(NG, 32, 16)      # [g, c, p16]
    a = a.transpose(2, 0, 1)                               # [p16, g, c]
    a = np.tile(a, (8, 1, 1))                              # [128, g, c]
    return np.ascontiguousarray(a.reshape(128, NG * 32))


def _make_in_maps(inputs):
    tokens = np.asarray(inputs["tokens"])
    h0 = np.asarray(inputs["h0"], dtype=np.float32)
    embedding = np.asarray(inputs["embedding"], dtype=np.float32)
    embb = embedding.astype(ml_dtypes.bfloat16)
    eye = np.eye(B, dtype=ml_dtypes.bfloat16)

    def wlay(w):
        wb = np.asarray(w, np.float32)[:, _PERM].astype(ml_dtypes.bfloat16)
        return np.ascontiguousarray(
            wb.reshape(4, 128, 2048).transpose(1, 0, 2).reshape(128, 8192))

    wxs = {0: wlay(inputs["wx_f"]), 1: wlay(inputs["wx_b"])}
    whs = {0: wlay(inputs["wh_f"]), 1: wlay(inputs["wh_b"])}
    bias = {}
    for d, (a, b) in enumerate((("bx_f", "bh_f"), ("bx_b", "bh_b"))):
        v = (np.asarray(inputs[a], np.float32) + np.asarray(inputs[b], np.float32))
        bias[d] = np.ascontiguousarray(
            v[_PERM].astype(ml_dtypes.bfloat16).reshape(1, 2048))

    in_maps = []
    for core in range(N_CORES):
        d = core // 4
        q = core % 4
        tok = tokens[:, q * B:(q + 1) * B]
        if d == 1:
            tok = tok[::-1]
        h0q = np.ascontiguousarray(h0[q * B:(q + 1) * B])   # [B, 512]
        h0T = np.ascontiguousarray(
            h0q.reshape(B, 4, 128).transpose(2, 1, 0).reshape(128, 4 * B))
        in_maps.append({
            "embb": embb,
            "idxs": _wrap_idxs(np.ascontiguousarray(tok).reshape(-1)),
            "wxs": wxs[d],
            "whs": whs[d],
            "biasb": bias[d],
            "h0T": h0T,
            "h0r": h0q,
            "eye8": eye,
        })
    return in_maps


def kernel(**inputs):
    import time
    from concourse.bass_utils import run_bass_kernel_spmd

    in_maps = _make_in_maps(inputs)
    nc = _get_nc()
    t0 = time.perf_counter()
    res = run_bass_kernel_spmd(nc, in_maps, list(range(N_CORES)))
    LAST_INFO["run_wall_s"] = time.perf_counter() - t0

    # ---- unshard: hist [B, S, 512] batch-major -> out [32, S*1024] ----
    out = np.empty((BATCH, S, 2, HID), np.float32)
    for core in range(N_CORES):
        d, q = core // 4, core % 4
        h = res.results[core]["hist"]                       # [B, S, 512]
        if d == 1:
            h = h[:, ::-1]
        out[q * B:(q + 1) * B, :, d, :] = h
    return np.ascontiguousarray(out.reshape(BATCH, S * 2 * HID))
